# revision 21
# baseline (speedup 1.0000x reference)
"""BetaGNN message-passing kernel for 8 Trainium2 NeuronCores.

Strategy (dest-row sharding, 6250 nodes/core):
  - Host relabels nodes: sorted by in-degree, dealt round-robin to cores so
    every core's tile t has near-identical max-degree -> uniform chunk counts.
  - Hop 1 (AH = A @ relu(x @ W_in^T + b)): no gather. Host pre-gathers the
    3-wide input features per edge (plus a ones column for the bias); the PE
    recomputes h per edge-slot with one K=4 bf16 matmul per 128-edge chunk.
    Edge values fold into the relu via per-partition scale; constant-identity
    matmuls accumulate chunk PAIRS (N=512) into per-tile PSUM; the two
    halves are summed by DVE at tile end.
  - AH (bf16) is AllGathered in 4 strided pieces, pipelined under phase A.
  - Hop 2 (A2H = A @ AH): single-row dma_gather (512B packets). int16 index
    range is handled by splitting each tile's chunks into two source-windows
    ([0,32768) and [17232,50000)) with separate table base offsets. A host-
    built selection*value matrix S (one nonzero per slot row) is the lhsT of
    one N=256 matmul per chunk: psum[col,:] += sum_p S[p,col]*AH[src_p,:].
  - Dense tail in transposed layout (PE transposes AH/A2H tiles, bf16):
    h2^T = relu(W1 AH^T + W2 A2H^T), g = softplus(W_out h2^T + b_out).
"""

import sys

for _p in ("/opt/trn_rl_repo", "/root/.axon_site/_ro/trn_rl_repo"):
    if _p not in sys.path:
        sys.path.insert(0, _p)

import numpy as np
import ml_dtypes

import concourse.bacc as bacc
import concourse.bass as bass
import concourse.mybir as mybir
from concourse import tile
from concourse.bass_utils import run_bass_kernel_spmd
from concourse import bass_utils as _bu

# Enable walrus LDWEIGHTS dedup (identity/weight tiles reused between
# matmuls; the default =false flag forces a reload per matmul).
_orig_gwa = _bu.get_walrus_args
def _gwa(*a, **k):
    return [str(x).replace("--enable-ldw-opt=false", "--enable-ldw-opt=true")
            for x in _orig_gwa(*a, **k)]
_bu.get_walrus_args = _gwa

F32 = mybir.dt.float32
BF16 = mybir.dt.bfloat16
I16 = mybir.dt.int16
AF = mybir.ActivationFunctionType

MAX_CALL_CHUNKS = 12      # <=12 chunks (1536 idxs) per dma_gather call
WIN = 32768               # int16-addressable window size
HI_BASE = None            # set per-P in Cfg (P - WIN, 0 if P <= WIN)


class Cfg:
    def __init__(self, P, E, nc=8, hid=256):
        assert P % (nc * 2) == 0
        self.P, self.E, self.NC, self.HID = P, E, nc, hid
        self.NPC = P // nc                    # nodes per core
        self.NT = (self.NPC + 127) // 128     # dest tiles per core
        self.NPAD = self.NT * 128
        self.HI_BASE = max(0, P - WIN)        # hi window = [HI_BASE, P)
        self.BLK = []
        off = 0
        while off < self.NPAD:
            w = min(512, self.NPAD - off)
            self.BLK.append((off, w))
            off += w
        # AllGather piece boundaries (in completed dest tiles). Each piece
        # writes a contiguous block of the table: rows off + c*R + (l - lo).
        npiece = 1  # DEBUG: single AG piece
        step = (self.NT + npiece - 1) // npiece
        self.AG_AT = []
        b = step
        while b < self.NT:
            self.AG_AT.append(b)
            b += step
        self.AG_AT.append(self.NT)
        self.PIECES = []
        off = 0
        lo = 0
        for bnd in self.AG_AT:
            hi = min(bnd * 128, self.NPC)
            self.PIECES.append((lo, hi, off))
            off += nc * (hi - lo)
            lo = hi
        assert off == P


def _plan(cfg, deg):
    P, NC, NT = cfg.P, cfg.NC, cfg.NT
    order = np.argsort(-deg, kind="stable")
    rank = np.empty(P, np.int64)
    rank[order] = np.arange(P)
    core_of = rank % NC
    local_of = rank // NC
    gid = core_of * cfg.NPC + local_of
    degs_sorted = deg[order]
    NCHUNK = []
    for t in range(NT):
        q = max(2, int(degs_sorted[t * 128 * NC]))
        NCHUNK.append(q + (q & 1))   # even, so acc-matmul chunk pairs
    NCHUNK = np.array(NCHUNK, np.int64)
    tile_off = np.concatenate([[0], np.cumsum(NCHUNK)])
    return core_of, local_of, gid, NCHUNK, tile_off, int(tile_off[-1])


def _prepare(cfg, beta, degree, A_rows, A_cols, A_vals,
             W_in, b_in, W_mp1, W_mp2, W_out, b_out):
    P, E, NC, NPC, NT = cfg.P, cfg.E, cfg.NC, cfg.NPC, cfg.NT
    deg = np.bincount(A_rows, minlength=P).astype(np.int64)
    core_of, local_of, gid, NCHUNK, tile_off, TC = _plan(cfg, deg)
    NSLOT = TC * 128

    d_gid = gid[A_rows.astype(np.int64)]
    oe = np.argsort(d_gid, kind="stable")
    sd = d_gid[oe]
    first = np.r_[True, sd[1:] != sd[:-1]]
    cumstart = np.maximum.accumulate(np.where(first, np.arange(E), 0))
    chunk = np.arange(E) - cumstart
    e_core = sd // NPC
    e_local = sd % NPC
    e_col = e_local % 128
    e_k = tile_off[e_local // 128] + chunk
    e_slot = e_k * 128 + e_col

    src = A_cols.astype(np.int64)[oe]
    vals = A_vals[oe].astype(np.float32)
    # table row of each node: piece-major AllGather layout
    row_of_gid = np.empty(P, np.int64)
    for (lo, hi, off) in cfg.PIECES:
        R = hi - lo
        for c in range(NC):
            row_of_gid[c * NPC + lo:c * NPC + hi] = (
                off + c * R + np.arange(R))
    sgid = row_of_gid[gid[src]]

    x4_all = np.stack([beta[:, 0], beta[:, 0] ** 2, degree[:, 0],
                       np.ones(P, np.float32)], axis=0).astype(np.float32)

    # ---- phase C chunk planning: per (core, tile) window split ----
    HI_BASE = cfg.HI_BASE
    # per core/tile edge index lists
    et_tile = e_local // 128
    lo_strict = sgid < HI_BASE           # must use lo window
    hi_strict = sgid >= WIN              # must use hi window
    a_min = np.zeros((NC, NT), np.int64)
    b_min = np.zeros((NC, NT), np.int64)
    n_ct = np.zeros((NC, NT), np.int64)
    np.add.at(n_ct, (e_core, et_tile), 1)
    np.add.at(a_min, (e_core[lo_strict], et_tile[lo_strict]), 1)
    np.add.at(b_min, (e_core[hi_strict], et_tile[hi_strict]), 1)
    C_lo = np.max(-(-a_min // 128), axis=0)        # per-tile across-core max
    C_hi = np.max(-(-b_min // 128), axis=0)
    need = np.max(-(-n_ct // 128), axis=0)
    # ensure capacity C_lo+C_hi >= need per tile, and at least one chunk
    bump = np.maximum(0, need - (C_lo + C_hi))
    C_hi = C_hi + bump
    C_lo = np.maximum(C_lo + C_hi, 1) - C_hi       # C_lo+C_hi >= 1
    C_lo = C_lo.astype(np.int64)
    C_hi = C_hi.astype(np.int64)
    TCC = int(np.sum(C_lo + C_hi))
    NSLOTC = TCC * 128
    # calls: per tile, lo chunks then hi chunks, <=MAX_CALL_CHUNKS per call
    callsC = []
    for t in range(NT):
        for win, cnt in ((0, int(C_lo[t])), (1, int(C_hi[t]))):
            rem = cnt
            while rem:
                g = min(MAX_CALL_CHUNKS, rem)
                callsC.append((t, win, g))
                rem -= g

    NIDXCOL = NSLOTC // 16
    per_core = []
    for c in range(NC):
        m = e_core == c
        # ---- phase A tensors (x4 quad-packed + v1), as baseline ----
        sl, km, cm = e_slot[m], e_k[m], e_col[m]
        x4T = np.zeros((4, NSLOT), np.float32)
        x4T[:, sl] = x4_all[:, src[m]]
        NQ = (TC + 3) // 4
        x4c = np.zeros((4, NQ * 4, 128), np.float32)
        x4c[:, :TC, :] = x4T.reshape(4, TC, 128)
        x4q = np.zeros((128, NQ * 128), np.float32)
        for j in range(4):
            x4q[32 * j:32 * j + 4, :] = (
                x4c[:, j::4, :].reshape(4, NQ * 128))
        v1 = np.zeros((128, TC), np.float32)
        v1[cm, km] = vals[m]

        # ---- phase C: window assignment, slots, S, idx ----
        tt_c = et_tile[m]
        sg_c = sgid[m]
        col_c = e_col[m]
        val_c = vals[m]
        idx_slot = np.zeros(NSLOTC, np.int16)
        s_mat = np.zeros((128, TCC, 128), ml_dtypes.bfloat16)
        kbase = 0
        for t in range(NT):
            sel = tt_c == t
            sg_t, col_t, val_t = sg_c[sel], col_c[sel], val_c[sel]
            n = len(sg_t)
            is_hi_strict = sg_t >= WIN
            is_lo_strict = sg_t < HI_BASE
            is_mid = ~is_hi_strict & ~is_lo_strict
            bm = int(np.sum(is_hi_strict))
            b = max(bm, n - int(C_lo[t]) * 128)
            a = n - b
            # lo set: all strict-lo + first (a - a_min) of mid
            amin = int(np.sum(is_lo_strict))
            nmid_lo = a - amin
            mid_idx = np.nonzero(is_mid)[0]
            lo_sel = np.zeros(n, bool)
            lo_sel[is_lo_strict] = True
            lo_sel[mid_idx[:nmid_lo]] = True
            for win, selw, cnt, base in (
                    (0, lo_sel, int(C_lo[t]), 0),
                    (1, ~lo_sel, int(C_hi[t]), HI_BASE)):
                nw = int(np.sum(selw))
                assert nw <= cnt * 128
                slots = kbase * 128 + np.arange(nw)
                idx_slot[slots] = (sg_t[selw] - base).astype(np.int16)
                p_in = np.arange(nw) % 128
                k_in = kbase + np.arange(nw) // 128
                s_mat[p_in, k_in, col_t[selw]] = val_t[selw].astype(
                    ml_dtypes.bfloat16)
                kbase += cnt
        assert kbase == TCC
        # pack indices per call ([16, ni/16] wrap, replicated x8)
        idxh = np.zeros((128, NIDXCOL), np.int16)
        col0 = 0
        soff = 0
        for (t, win, g) in callsC:
            ni = g * 128
            blockv = idx_slot[soff:soff + ni].reshape(ni // 16, 16).T
            for q in range(8):
                idxh[16 * q:16 * (q + 1), col0:col0 + ni // 16] = blockv
            col0 += ni // 16
            soff += ni
        per_core.append(dict(
            x4q=x4q.astype(ml_dtypes.bfloat16),
            v1=v1,
            sc=s_mat.reshape(128, TCC * 128),
            idx=idxh))

    wiT = np.concatenate([W_in.T.astype(np.float32),
                          b_in[None, :].astype(np.float32)], axis=0)
    wiT4 = np.zeros((128, wiT.shape[1]), np.float32)
    for j in range(4):
        wiT4[32 * j:32 * j + 4, :] = wiT
    consts = dict(
        wit=wiT4.astype(ml_dtypes.bfloat16),
        w1t=np.ascontiguousarray(W_mp1.T).astype(ml_dtypes.bfloat16),
        w2t=np.ascontiguousarray(W_mp2.T).astype(ml_dtypes.bfloat16),
        wot=np.ascontiguousarray(W_out.T).astype(ml_dtypes.bfloat16),
        bout=np.full((128, 1), float(np.asarray(b_out).reshape(-1)[0]),
                     np.float32),
        idn16=np.eye(128, dtype=np.float32).astype(ml_dtypes.bfloat16),
    )
    meta = dict(NCHUNK=tuple(int(x) for x in NCHUNK),
                C_lo=tuple(int(x) for x in C_lo),
                C_hi=tuple(int(x) for x in C_hi),
                callsC=tuple(callsC),
                TC=TC, TCC=TCC, NIDXCOL=NIDXCOL, NQ=(TC + 3) // 4)
    return per_core, consts, meta, (core_of, local_of)


def _build(cfg, meta):
    NT, NPC, NPAD, HID, NC, P = (cfg.NT, cfg.NPC, cfg.NPAD, cfg.HID,
                                 cfg.NC, cfg.P)
    NCHUNK = meta["NCHUNK"]
    C_lo, C_hi, callsC = meta["C_lo"], meta["C_hi"], meta["callsC"]
    TC, TCC, NIDXCOL, NQ = meta["TC"], meta["TCC"], meta["NIDXCOL"], meta["NQ"]
    tile_off = np.concatenate([[0], np.cumsum(NCHUNK)])
    NBLK = len(cfg.BLK)

    nc = bacc.Bacc("TRN2", target_bir_lowering=False, debug=False)
    x4T_d = nc.dram_tensor("x4t", [128, NQ * 128], BF16, kind="ExternalInput")
    v1_d = nc.dram_tensor("v1", [128, TC], F32, kind="ExternalInput")
    sc_d = nc.dram_tensor("sc", [128, TCC * 128], BF16, kind="ExternalInput")
    idx_d = nc.dram_tensor("idx", [128, NIDXCOL], I16, kind="ExternalInput")
    wiT_d = nc.dram_tensor("wit", [128, HID], BF16, kind="ExternalInput")
    w1T_d = nc.dram_tensor("w1t", [HID, HID], BF16, kind="ExternalInput")
    w2T_d = nc.dram_tensor("w2t", [HID, HID], BF16, kind="ExternalInput")
    woT_d = nc.dram_tensor("wot", [HID, 1], BF16, kind="ExternalInput")
    bout_d = nc.dram_tensor("bout", [128, 1], F32, kind="ExternalInput")
    idn16_d = nc.dram_tensor("idn16", [128, 128], BF16, kind="ExternalInput")
    g_d = nc.dram_tensor("g", [1, NBLK * 512], F32, kind="ExternalOutput")

    ah_bounce = nc.dram_tensor("ah_bounce", [NPC, HID], BF16)
    ah_full = nc.dram_tensor("ah_full", [P, HID], BF16, addr_space="Shared")

    with tile.TileContext(nc) as tc:
        with (
            tc.tile_pool(name="const", bufs=1) as constp,
            tc.tile_pool(name="xs", bufs=3) as xsp,
            tc.tile_pool(name="msgs", bufs=12) as msgp,
            tc.tile_pool(name="stage", bufs=3) as stagep,
            tc.tile_pool(name="resid", bufs=1) as residp,
            tc.tile_pool(name="pair", bufs=4) as pairp,
            tc.tile_pool(name="ph", bufs=4, space="PSUM") as php,
            tc.tile_pool(name="pz", bufs=2, space="PSUM") as pzp,
            tc.tile_pool(name="pt", bufs=2, space="PSUM") as ptp,
        ):
            wiT = constp.tile([128, HID], BF16, tag="wiT", name="wiT")
            nc.sync.dma_start(wiT[:], wiT_d[:])
            w1T = [constp.tile([128, HID], BF16, tag=f"w1_{k}", name=f"w1_{k}")
                   for k in (0, 1)]
            w2T = [constp.tile([128, HID], BF16, tag=f"w2_{k}", name=f"w2_{k}")
                   for k in (0, 1)]
            for k in (0, 1):
                nc.sync.dma_start(w1T[k][:], w1T_d[128 * k:128 * (k + 1), :])
                nc.sync.dma_start(w2T[k][:], w2T_d[128 * k:128 * (k + 1), :])
            woT = constp.tile([128, 2], BF16, tag="woT", name="woT")
            nc.sync.dma_start(woT[:, 0:1], woT_d[0:128, :])
            nc.sync.dma_start(woT[:, 1:2], woT_d[128:256, :])
            bout = constp.tile([128, 1], F32, tag="bout", name="bout")
            nc.sync.dma_start(bout[:], bout_d[:])
            idn16 = constp.tile([128, 128], BF16, tag="idn16", name="idn16")
            nc.sync.dma_start(idn16[:], idn16_d[:])
            v1 = constp.tile([128, TC], F32, tag="v1", name="v1")
            nc.sync.dma_start(v1[:], v1_d[:])
            idx = constp.tile([128, NIDXCOL], I16, tag="idx", name="idx")
            nc.sync.dma_start(idx[:], idx_d[:])

            ahT = [residp.tile([128, NPAD], BF16, tag=f"ahT{m}", name=f"ahT{m}")
                   for m in (0, 1)]
            a2T = [residp.tile([128, NPAD], BF16, tag=f"a2T{m}", name=f"a2T{m}")
                   for m in (0, 1)]

            def issue_ag(piece):
                # AllGather local rows [lo, hi) into the contiguous table
                # block [off, off + NC*(hi-lo)): replica c lands at off + c*R.
                (lo, hi, off) = cfg.PIECES[piece]
                R = hi - lo
                nc.gpsimd.collective_compute(
                    "AllGather", mybir.AluOpType.bypass,
                    replica_groups=[list(range(NC))],
                    ins=[ah_bounce[lo:hi, :]],
                    outs=[ah_full[off:off + NC * R, :]],
                )

            # ---- phase A: hop 1 (quad-packed K=4 matmuls, groups of 8) ----
            # software pipeline: acc matmuls run one group behind the
            # h-matmuls so relu latency is hidden.
            state = dict(t=0, pz=None, pend=[], half=None, ag=0)

            def epilogue_a(tt, pzv):
                # combine pair halves, emit bf16 AH tile + transposes
                tmp = stagep.tile([128, HID], BF16, tag="tmp", name="tmp")
                nc.vector.tensor_copy(tmp[:], pzv[:, HID:2 * HID])
                ahb = stagep.tile([128, HID], BF16, tag="ahb", name="ahb")
                nc.vector.tensor_tensor(
                    ahb[:], pzv[:, :HID], tmp[:],
                    op=mybir.AluOpType.add)
                rows = min(128, NPC - tt * 128)
                nc.sync.dma_start(ah_bounce[tt * 128:tt * 128 + rows, :],
                                  ahb[:rows, :])
                for mh in (0, 1):
                    pt = ptp.tile([128, 1024], BF16, tag="pt", name="pt")
                    nc.tensor.transpose(
                        pt[:, :128], ahb[:, mh * 128:(mh + 1) * 128],
                        idn16[:])
                    nc.vector.tensor_copy(
                        ahT[mh][:, tt * 128:(tt + 1) * 128], pt[:, :128])
                for j, bnd in enumerate(cfg.AG_AT):
                    if tt + 1 == bnd:
                        issue_ag(j)
                        state["ag"] = j + 1

            def flush_one():
                # consume one pending chunk-pair into the accumulator psum;
                # advance tile state. NCHUNK is even so pairs never span
                # tiles and both psum halves are always started/stopped.
                k0, m2 = state["pend"].pop(0)
                t = state["t"]
                if k0 == int(tile_off[t]):
                    state["pz"] = pzp.tile([128, 512], F32, tag="acc",
                                           name="acc")
                pz = state["pz"]
                last = int(tile_off[t + 1]) - 1
                nc.tensor.matmul(
                    pz[:], lhsT=idn16[:], rhs=m2[:],
                    start=(k0 == int(tile_off[t])),
                    stop=(k0 + 1 == last),
                    skip_group_check=True)
                if k0 + 1 == last:
                    epilogue_a(t, pz)
                    state["t"] = t + 1

            t = 0
            for g8 in range(0, TC, 8):
                khi = min(g8 + 8, TC)
                xs = xsp.tile([128, 2 * 128], BF16, tag="xs", name="xs")
                q0 = g8 // 4
                hi = min((q0 + 2) * 128, NQ * 128)
                nc.sync.dma_start(xs[:, :hi - q0 * 128],
                                  x4T_d[:, q0 * 128:hi])
                phs = []
                for k in range(g8, khi):
                    j, half = k % 4, (k - g8) // 4
                    ph = php.tile([128, 512], F32, tag="ph", name="ph")
                    nc.tensor.matmul(
                        ph[:, :HID],
                        lhsT=xs[32 * j:32 * j + 4,
                                half * 128:(half + 1) * 128],
                        rhs=wiT[32 * j:32 * j + 4, :],
                        start=True, stop=True, skip_group_check=True,
                        tile_position=(32 * j, 0))
                    phs.append(ph)
                # relus write chunk pairs into halves of a shared m2 tile;
                # even NCHUNK means pairs are (even k, k+1) and never span
                # a dest tile.
                for k in range(g8, khi):
                    ph = phs[k - g8]
                    if k % 2 == 0:
                        m2 = msgp.tile([128, 2 * HID], BF16, tag="m2",
                                       name="m2")
                        state["half"] = (m2, k)
                        nc.scalar.activation(m2[:, 0:HID], ph[:, :HID],
                                             AF.Relu, scale=v1[:, k:k + 1])
                    else:
                        m2, k0 = state["half"]
                        nc.vector.tensor_scalar(
                            m2[:, HID:2 * HID], ph[:, :HID],
                            v1[:, k:k + 1], 0.0,
                            op0=mybir.AluOpType.mult,
                            op1=mybir.AluOpType.max)
                        state["pend"].append((k0, m2))
                        state["half"] = None
                # flush pending pairs except those from the current group
                while len(state["pend"]) > 4:
                    flush_one()
            while state["pend"]:
                flush_one()
            while state["ag"] < len(cfg.PIECES):
                issue_ag(state["ag"])
                state["ag"] += 1

            ah_lo = ah_full[0:min(P, WIN), :]
            ah_hi = ah_full[cfg.HI_BASE:P, :]

            # ---- phase C: hop 2 (single-row gathers + S matmuls) ----
            ci = 0
            col0 = 0
            sk = 0
            for t in range(NT):
                ncht = int(C_lo[t]) + int(C_hi[t])
                pz = pzp.tile([128, 512], F32, tag="acc", name="acc")
                done = 0
                while done < ncht:
                    (tt, win, g) = callsC[ci]
                    assert tt == t
                    ni = g * 128
                    pr = pairp.tile([128, MAX_CALL_CHUNKS, HID], BF16,
                                    tag="pair", name="pair")
                    nc.gpsimd.dma_gather(
                        pr[:, :g, :], ah_lo if win == 0 else ah_hi,
                        idx[:, col0:col0 + ni // 16],
                        ni, ni, HID, single_packet=False)
                    sdl = msgp.tile([128, MAX_CALL_CHUNKS * 128], BF16,
                                    tag="sdl", name="sdl", bufs=3)
                    nc.sync.dma_start(sdl[:, :ni],
                                      sc_d[:, sk * 128:sk * 128 + ni])
                    for cc in range(g):
                        nc.tensor.matmul(
                            pz[:, :HID],
                            lhsT=sdl[:, cc * 128:(cc + 1) * 128],
                            rhs=pr[:, cc, :],
                            start=(done + cc == 0),
                            stop=(done + cc == ncht - 1),
                            skip_group_check=True)
                    done += g
                    sk += g
                    col0 += ni // 16
                    ci += 1
                a2b = stagep.tile([128, HID], BF16, tag="a2b", name="a2b")
                nc.vector.tensor_copy(a2b[:], pz[:, :HID])
                for mh in (0, 1):
                    pt = ptp.tile([128, 1024], BF16, tag="pt", name="pt")
                    nc.tensor.transpose(
                        pt[:, :128], a2b[:, mh * 128:(mh + 1) * 128],
                        idn16[:])
                    nc.vector.tensor_copy(
                        a2T[mh][:, t * 128:(t + 1) * 128], pt[:, :128])

            # ---- phase D: dense tail ----
            for b, (off, w) in enumerate(cfg.BLK):
                h2 = []
                for mh in (0, 1):
                    pd = pzp.tile([128, 512], F32, tag="acc", name="acc")
                    n = 0
                    for (wt, xt) in ((w1T, ahT), (w2T, a2T)):
                        for k in (0, 1):
                            nc.tensor.matmul(
                                pd[:, :w],
                                lhsT=wt[k][:, mh * 128:(mh + 1) * 128],
                                rhs=xt[k][:, off:off + w],
                                start=(n == 0), stop=(n == 3),
                                skip_group_check=True)
                            n += 1
                    ht = stagep.tile([128, 512], BF16, tag="h2t", name="h2t")
                    nc.scalar.activation(ht[:, :w], pd[:, :w], AF.Relu)
                    h2.append(ht)
                pg = php.tile([1, 512], F32, tag="ph", name="pg")
                for k in (0, 1):
                    nc.tensor.matmul(pg[:, :w],
                                     lhsT=woT[:, k:k + 1],
                                     rhs=h2[k][:, :w],
                                     start=(k == 0), stop=(k == 1),
                                     skip_group_check=True)
                gb = stagep.tile([1, 512], F32, tag="gbuf", name="gb",
                                 bufs=4)
                nc.vector.tensor_copy(gb[0:1, :w], pg[:, :w])
                ge = stagep.tile([1, 512], F32, tag="gbuf", name="ge",
                                 bufs=4)
                nc.scalar.activation(ge[0:1, :w], gb[0:1, :w], AF.Exp,
                                     bias=bout[0:1, :])
                go = stagep.tile([1, 512], F32, tag="gbuf", name="go",
                                 bufs=4)
                nc.scalar.activation(go[0:1, :w], ge[0:1, :w], AF.Ln,
                                     bias=1.0)
                nc.sync.dma_start(g_d[0:1, off:off + w], go[0:1, :w])

    nc.compile()
    return nc


_COMPILED = {}


def _get_compiled(cfg, meta):
    key = (cfg.P, cfg.E, meta["NCHUNK"], meta["C_lo"], meta["C_hi"],
           meta["callsC"])
    if key not in _COMPILED:
        _COMPILED[key] = _build(cfg, meta)
    return _COMPILED[key]


def run(cfg, inputs, trace=False):
    per_core, consts, meta, (core_of, local_of) = _prepare(cfg, **inputs)
    ncobj = _get_compiled(cfg, meta)
    in_maps = []
    for c in range(cfg.NC):
        pc = per_core[c]
        im = {"x4t": pc["x4q"], "v1": pc["v1"], "sc": pc["sc"],
              "idx": pc["idx"]}
        im.update({k: np.asarray(v) for k, v in consts.items()})
        in_maps.append(im)
    res = run_bass_kernel_spmd(ncobj, in_maps, list(range(cfg.NC)),
                               trace=trace)
    g = np.empty(cfg.P, np.float32)
    for c in range(cfg.NC):
        go = np.asarray(res.results[c]["g"]).reshape(-1)
        mine = core_of == c
        g[mine] = go[local_of[mine]]
    return g.reshape(cfg.P, 1), res


def kernel(**inputs):
    cfg = Cfg(P=50000, E=800000)
    g, _ = run(cfg, inputs)
    return g


# revision 22
# speedup vs baseline: 1.0506x; 1.0506x over previous
"""BetaGNN message-passing kernel for 8 Trainium2 NeuronCores.

Strategy (dest-row sharding, 6250 nodes/core):
  - Host relabels nodes: sorted by in-degree, dealt round-robin to cores so
    every core's tile t has near-identical max-degree -> uniform chunk counts.
  - Hop 1 (AH = A @ relu(x @ W_in^T + b)): no gather. Host pre-gathers the
    3-wide input features per edge (plus a ones column for the bias); the PE
    recomputes h per edge-slot with one K=4 bf16 matmul per 128-edge chunk.
    Edge values fold into the relu via per-partition scale; constant-identity
    matmuls accumulate chunk PAIRS (N=512) into per-tile PSUM; the two
    halves are summed by DVE at tile end.
  - AH (bf16) is AllGathered in 4 strided pieces, pipelined under phase A.
  - Hop 2 (A2H = A @ AH): single-row dma_gather (512B packets). int16 index
    range is handled by splitting each tile's chunks into two source-windows
    ([0,32768) and [17232,50000)) with separate table base offsets. A host-
    built selection*value matrix S (one nonzero per slot row) is the lhsT of
    one N=256 matmul per chunk: psum[col,:] += sum_p S[p,col]*AH[src_p,:].
  - Dense tail in transposed layout (PE transposes AH/A2H tiles, bf16):
    h2^T = relu(W1 AH^T + W2 A2H^T), g = softplus(W_out h2^T + b_out).
"""

import sys

for _p in ("/opt/trn_rl_repo", "/root/.axon_site/_ro/trn_rl_repo"):
    if _p not in sys.path:
        sys.path.insert(0, _p)

import numpy as np
import ml_dtypes

import concourse.bacc as bacc
import concourse.bass as bass
import concourse.mybir as mybir
from concourse import tile
from concourse.bass_utils import run_bass_kernel_spmd
from concourse import bass_utils as _bu

# Enable walrus LDWEIGHTS dedup (identity/weight tiles reused between
# matmuls; the default =false flag forces a reload per matmul).
_orig_gwa = _bu.get_walrus_args
def _gwa(*a, **k):
    return [str(x).replace("--enable-ldw-opt=false", "--enable-ldw-opt=true")
            for x in _orig_gwa(*a, **k)]
_bu.get_walrus_args = _gwa

F32 = mybir.dt.float32
BF16 = mybir.dt.bfloat16
I16 = mybir.dt.int16
AF = mybir.ActivationFunctionType

DEBUG_DUMP = False
MAX_CALL_CHUNKS = 12      # <=12 chunks (1536 idxs) per dma_gather call
WIN = 32768               # int16-addressable window size
HI_BASE = None            # set per-P in Cfg (P - WIN, 0 if P <= WIN)


class Cfg:
    def __init__(self, P, E, nc=8, hid=256):
        assert P % (nc * 2) == 0
        self.P, self.E, self.NC, self.HID = P, E, nc, hid
        self.NPC = P // nc                    # nodes per core
        self.NT = (self.NPC + 127) // 128     # dest tiles per core
        self.NPAD = self.NT * 128
        self.HI_BASE = max(0, P - WIN)        # hi window = [HI_BASE, P)
        self.BLK = []
        off = 0
        while off < self.NPAD:
            w = min(512, self.NPAD - off)
            self.BLK.append((off, w))
            off += w
        # AllGather piece boundaries (in completed dest tiles). Each piece
        # writes a contiguous block of the table: rows off + c*R + (l - lo).
        npiece = 4 if self.NT >= 8 else 1
        step = (self.NT + npiece - 1) // npiece
        self.AG_AT = []
        b = step
        while b < self.NT:
            self.AG_AT.append(b)
            b += step
        self.AG_AT.append(self.NT)
        self.PIECES = []
        off = 0
        lo = 0
        for bnd in self.AG_AT:
            hi = min(bnd * 128, self.NPC)
            self.PIECES.append((lo, hi, off))
            off += nc * (hi - lo)
            lo = hi
        assert off == P


def _plan(cfg, deg):
    P, NC, NT = cfg.P, cfg.NC, cfg.NT
    order = np.argsort(-deg, kind="stable")
    rank = np.empty(P, np.int64)
    rank[order] = np.arange(P)
    core_of = rank % NC
    local_of = rank // NC
    gid = core_of * cfg.NPC + local_of
    degs_sorted = deg[order]
    NCHUNK = []
    for t in range(NT):
        q = max(2, int(degs_sorted[t * 128 * NC]))
        NCHUNK.append(q + (q & 1))   # even, so acc-matmul chunk pairs
    NCHUNK = np.array(NCHUNK, np.int64)
    tile_off = np.concatenate([[0], np.cumsum(NCHUNK)])
    return core_of, local_of, gid, NCHUNK, tile_off, int(tile_off[-1])


def _prepare(cfg, beta, degree, A_rows, A_cols, A_vals,
             W_in, b_in, W_mp1, W_mp2, W_out, b_out):
    P, E, NC, NPC, NT = cfg.P, cfg.E, cfg.NC, cfg.NPC, cfg.NT
    deg = np.bincount(A_rows, minlength=P).astype(np.int64)
    core_of, local_of, gid, NCHUNK, tile_off, TC = _plan(cfg, deg)
    NSLOT = TC * 128

    d_gid = gid[A_rows.astype(np.int64)]
    oe = np.argsort(d_gid, kind="stable")
    sd = d_gid[oe]
    first = np.r_[True, sd[1:] != sd[:-1]]
    cumstart = np.maximum.accumulate(np.where(first, np.arange(E), 0))
    chunk = np.arange(E) - cumstart
    e_core = sd // NPC
    e_local = sd % NPC
    e_col = e_local % 128
    e_k = tile_off[e_local // 128] + chunk
    e_slot = e_k * 128 + e_col

    src = A_cols.astype(np.int64)[oe]
    vals = A_vals[oe].astype(np.float32)
    # table row of each node: piece-major AllGather layout
    row_of_gid = np.empty(P, np.int64)
    for (lo, hi, off) in cfg.PIECES:
        R = hi - lo
        for c in range(NC):
            row_of_gid[c * NPC + lo:c * NPC + hi] = (
                off + c * R + np.arange(R))
    sgid = row_of_gid[gid[src]]

    x4_all = np.stack([beta[:, 0], beta[:, 0] ** 2, degree[:, 0],
                       np.ones(P, np.float32)], axis=0).astype(np.float32)

    # ---- phase C chunk planning: per (core, tile) window split ----
    HI_BASE = cfg.HI_BASE
    # per core/tile edge index lists
    et_tile = e_local // 128
    lo_strict = sgid < HI_BASE           # must use lo window
    hi_strict = sgid >= WIN              # must use hi window
    a_min = np.zeros((NC, NT), np.int64)
    b_min = np.zeros((NC, NT), np.int64)
    n_ct = np.zeros((NC, NT), np.int64)
    np.add.at(n_ct, (e_core, et_tile), 1)
    np.add.at(a_min, (e_core[lo_strict], et_tile[lo_strict]), 1)
    np.add.at(b_min, (e_core[hi_strict], et_tile[hi_strict]), 1)
    C_lo = np.max(-(-a_min // 128), axis=0)        # per-tile across-core max
    C_hi = np.max(-(-b_min // 128), axis=0)
    need = np.max(-(-n_ct // 128), axis=0)
    # ensure capacity C_lo+C_hi >= need per tile, and at least one chunk
    bump = np.maximum(0, need - (C_lo + C_hi))
    C_hi = C_hi + bump
    C_lo = np.maximum(C_lo + C_hi, 1) - C_hi       # C_lo+C_hi >= 1
    C_lo = C_lo.astype(np.int64)
    C_hi = C_hi.astype(np.int64)
    TCC = int(np.sum(C_lo + C_hi))
    NSLOTC = TCC * 128
    # calls: per tile, lo chunks then hi chunks, <=MAX_CALL_CHUNKS per call
    callsC = []
    for t in range(NT):
        for win, cnt in ((0, int(C_lo[t])), (1, int(C_hi[t]))):
            rem = cnt
            while rem:
                g = min(MAX_CALL_CHUNKS, rem)
                callsC.append((t, win, g))
                rem -= g

    NIDXCOL = NSLOTC // 16
    per_core = []
    for c in range(NC):
        m = e_core == c
        # ---- phase A tensors (x4 quad-packed + v1), as baseline ----
        sl, km, cm = e_slot[m], e_k[m], e_col[m]
        x4T = np.zeros((4, NSLOT), np.float32)
        x4T[:, sl] = x4_all[:, src[m]]
        NQ = (TC + 3) // 4
        x4c = np.zeros((4, NQ * 4, 128), np.float32)
        x4c[:, :TC, :] = x4T.reshape(4, TC, 128)
        x4q = np.zeros((128, NQ * 128), np.float32)
        for j in range(4):
            x4q[32 * j:32 * j + 4, :] = (
                x4c[:, j::4, :].reshape(4, NQ * 128))
        v1 = np.zeros((128, TC), np.float32)
        v1[cm, km] = vals[m]

        # ---- phase C: window assignment, slots, S, idx ----
        tt_c = et_tile[m]
        sg_c = sgid[m]
        col_c = e_col[m]
        val_c = vals[m]
        idx_slot = np.zeros(NSLOTC, np.int16)
        s_mat = np.zeros((128, TCC, 128), ml_dtypes.bfloat16)
        kbase = 0
        for t in range(NT):
            sel = tt_c == t
            sg_t, col_t, val_t = sg_c[sel], col_c[sel], val_c[sel]
            n = len(sg_t)
            is_hi_strict = sg_t >= WIN
            is_lo_strict = sg_t < HI_BASE
            is_mid = ~is_hi_strict & ~is_lo_strict
            bm = int(np.sum(is_hi_strict))
            b = max(bm, n - int(C_lo[t]) * 128)
            a = n - b
            # lo set: all strict-lo + first (a - a_min) of mid
            amin = int(np.sum(is_lo_strict))
            nmid_lo = a - amin
            mid_idx = np.nonzero(is_mid)[0]
            lo_sel = np.zeros(n, bool)
            lo_sel[is_lo_strict] = True
            lo_sel[mid_idx[:nmid_lo]] = True
            for win, selw, cnt, base in (
                    (0, lo_sel, int(C_lo[t]), 0),
                    (1, ~lo_sel, int(C_hi[t]), HI_BASE)):
                nw = int(np.sum(selw))
                assert nw <= cnt * 128
                slots = kbase * 128 + np.arange(nw)
                idx_slot[slots] = (sg_t[selw] - base).astype(np.int16)
                p_in = np.arange(nw) % 128
                k_in = kbase + np.arange(nw) // 128
                s_mat[p_in, k_in, col_t[selw]] = val_t[selw].astype(
                    ml_dtypes.bfloat16)
                kbase += cnt
        assert kbase == TCC
        # pack indices per call ([16, ni/16] wrap, replicated x8)
        idxh = np.zeros((128, NIDXCOL), np.int16)
        col0 = 0
        soff = 0
        for (t, win, g) in callsC:
            ni = g * 128
            blockv = idx_slot[soff:soff + ni].reshape(ni // 16, 16).T
            for q in range(8):
                idxh[16 * q:16 * (q + 1), col0:col0 + ni // 16] = blockv
            col0 += ni // 16
            soff += ni
        per_core.append(dict(
            x4q=x4q.astype(ml_dtypes.bfloat16),
            v1=v1,
            sc=s_mat.reshape(128, TCC * 128),
            idx=idxh))

    wiT = np.concatenate([W_in.T.astype(np.float32),
                          b_in[None, :].astype(np.float32)], axis=0)
    wiT4 = np.zeros((128, wiT.shape[1]), np.float32)
    for j in range(4):
        wiT4[32 * j:32 * j + 4, :] = wiT
    consts = dict(
        wit=wiT4.astype(ml_dtypes.bfloat16),
        w1t=np.ascontiguousarray(W_mp1.T).astype(ml_dtypes.bfloat16),
        w2t=np.ascontiguousarray(W_mp2.T).astype(ml_dtypes.bfloat16),
        wot=np.ascontiguousarray(W_out.T).astype(ml_dtypes.bfloat16),
        bout=np.full((128, 1), float(np.asarray(b_out).reshape(-1)[0]),
                     np.float32),
        idn16=np.eye(128, dtype=np.float32).astype(ml_dtypes.bfloat16),
    )
    meta = dict(NCHUNK=tuple(int(x) for x in NCHUNK),
                C_lo=tuple(int(x) for x in C_lo),
                C_hi=tuple(int(x) for x in C_hi),
                callsC=tuple(callsC),
                TC=TC, TCC=TCC, NIDXCOL=NIDXCOL, NQ=(TC + 3) // 4)
    return per_core, consts, meta, (core_of, local_of)


def _build(cfg, meta):
    NT, NPC, NPAD, HID, NC, P = (cfg.NT, cfg.NPC, cfg.NPAD, cfg.HID,
                                 cfg.NC, cfg.P)
    NCHUNK = meta["NCHUNK"]
    C_lo, C_hi, callsC = meta["C_lo"], meta["C_hi"], meta["callsC"]
    TC, TCC, NIDXCOL, NQ = meta["TC"], meta["TCC"], meta["NIDXCOL"], meta["NQ"]
    tile_off = np.concatenate([[0], np.cumsum(NCHUNK)])
    NBLK = len(cfg.BLK)

    nc = bacc.Bacc("TRN2", target_bir_lowering=False, debug=False)
    x4T_d = nc.dram_tensor("x4t", [128, NQ * 128], BF16, kind="ExternalInput")
    v1_d = nc.dram_tensor("v1", [128, TC], F32, kind="ExternalInput")
    sc_d = nc.dram_tensor("sc", [128, TCC * 128], BF16, kind="ExternalInput")
    idx_d = nc.dram_tensor("idx", [128, NIDXCOL], I16, kind="ExternalInput")
    wiT_d = nc.dram_tensor("wit", [128, HID], BF16, kind="ExternalInput")
    w1T_d = nc.dram_tensor("w1t", [HID, HID], BF16, kind="ExternalInput")
    w2T_d = nc.dram_tensor("w2t", [HID, HID], BF16, kind="ExternalInput")
    woT_d = nc.dram_tensor("wot", [HID, 1], BF16, kind="ExternalInput")
    bout_d = nc.dram_tensor("bout", [128, 1], F32, kind="ExternalInput")
    idn16_d = nc.dram_tensor("idn16", [128, 128], BF16, kind="ExternalInput")
    g_d = nc.dram_tensor("g", [1, NBLK * 512], F32, kind="ExternalOutput")
    ahdump_d = (nc.dram_tensor("ahdump", [P, HID], BF16,
                kind="ExternalOutput") if DEBUG_DUMP else None)

    ah_bounce = nc.dram_tensor("ah_bounce", [NPC, HID], BF16)
    ah_full = nc.dram_tensor("ah_full", [P, HID], BF16, addr_space="Shared")

    with tile.TileContext(nc) as tc:
        with (
            tc.tile_pool(name="const", bufs=1) as constp,
            tc.tile_pool(name="xs", bufs=3) as xsp,
            tc.tile_pool(name="msgs", bufs=12) as msgp,
            tc.tile_pool(name="stage", bufs=3) as stagep,
            tc.tile_pool(name="resid", bufs=1) as residp,
            tc.tile_pool(name="pair", bufs=4) as pairp,
            tc.tile_pool(name="ph", bufs=4, space="PSUM") as php,
            tc.tile_pool(name="pz", bufs=2, space="PSUM") as pzp,
            tc.tile_pool(name="pt", bufs=2, space="PSUM") as ptp,
        ):
            wiT = constp.tile([128, HID], BF16, tag="wiT", name="wiT")
            nc.sync.dma_start(wiT[:], wiT_d[:])
            w1T = [constp.tile([128, HID], BF16, tag=f"w1_{k}", name=f"w1_{k}")
                   for k in (0, 1)]
            w2T = [constp.tile([128, HID], BF16, tag=f"w2_{k}", name=f"w2_{k}")
                   for k in (0, 1)]
            for k in (0, 1):
                nc.sync.dma_start(w1T[k][:], w1T_d[128 * k:128 * (k + 1), :])
                nc.sync.dma_start(w2T[k][:], w2T_d[128 * k:128 * (k + 1), :])
            woT = constp.tile([128, 2], BF16, tag="woT", name="woT")
            nc.sync.dma_start(woT[:, 0:1], woT_d[0:128, :])
            nc.sync.dma_start(woT[:, 1:2], woT_d[128:256, :])
            bout = constp.tile([128, 1], F32, tag="bout", name="bout")
            nc.sync.dma_start(bout[:], bout_d[:])
            idn16 = constp.tile([128, 128], BF16, tag="idn16", name="idn16")
            nc.sync.dma_start(idn16[:], idn16_d[:])
            v1 = constp.tile([128, TC], F32, tag="v1", name="v1")
            nc.sync.dma_start(v1[:], v1_d[:])
            idx = constp.tile([128, NIDXCOL], I16, tag="idx", name="idx")
            nc.sync.dma_start(idx[:], idx_d[:])

            ahT = [residp.tile([128, NPAD], BF16, tag=f"ahT{m}", name=f"ahT{m}")
                   for m in (0, 1)]
            a2T = [residp.tile([128, NPAD], BF16, tag=f"a2T{m}", name=f"a2T{m}")
                   for m in (0, 1)]

            def issue_ag(piece):
                # AllGather local rows [lo, hi) into the contiguous table
                # block [off, off + NC*(hi-lo)): replica c lands at off + c*R.
                (lo, hi, off) = cfg.PIECES[piece]
                R = hi - lo
                nc.gpsimd.collective_compute(
                    "AllGather", mybir.AluOpType.bypass,
                    replica_groups=[list(range(NC))],
                    ins=[ah_bounce[lo:hi, :]],
                    outs=[ah_full[off:off + NC * R, :]],
                )

            # ---- phase A: hop 1 (quad-packed K=4 matmuls, groups of 8) ----
            # software pipeline: acc matmuls run one group behind the
            # h-matmuls so relu latency is hidden.
            state = dict(t=0, pz=None, pend=[], half=None, ag=0)

            def epilogue_a(tt, pzv):
                # combine pair halves, emit bf16 AH tile + transposes
                tmp = stagep.tile([128, HID], BF16, tag="tmp", name="tmp")
                nc.vector.tensor_copy(tmp[:], pzv[:, HID:2 * HID])
                ahb = stagep.tile([128, HID], BF16, tag="ahb", name="ahb")
                nc.vector.tensor_tensor(
                    ahb[:], pzv[:, :HID], tmp[:],
                    op=mybir.AluOpType.add)
                rows = min(128, NPC - tt * 128)
                nc.sync.dma_start(ah_bounce[tt * 128:tt * 128 + rows, :],
                                  ahb[:rows, :])
                for mh in (0, 1):
                    pt = ptp.tile([128, 1024], BF16, tag="pt", name="pt")
                    nc.tensor.transpose(
                        pt[:, :128], ahb[:, mh * 128:(mh + 1) * 128],
                        idn16[:])
                    nc.vector.tensor_copy(
                        ahT[mh][:, tt * 128:(tt + 1) * 128], pt[:, :128])
                for j, bnd in enumerate(cfg.AG_AT):
                    if tt + 1 == bnd:
                        issue_ag(j)
                        state["ag"] = j + 1

            def flush_one():
                # consume one pending chunk-pair into the accumulator psum;
                # advance tile state. NCHUNK is even so pairs never span
                # tiles and both psum halves are always started/stopped.
                k0, m2 = state["pend"].pop(0)
                t = state["t"]
                if k0 == int(tile_off[t]):
                    state["pz"] = pzp.tile([128, 512], F32, tag="acc",
                                           name="acc")
                pz = state["pz"]
                last = int(tile_off[t + 1]) - 1
                nc.tensor.matmul(
                    pz[:], lhsT=idn16[:], rhs=m2[:],
                    start=(k0 == int(tile_off[t])),
                    stop=(k0 + 1 == last),
                    skip_group_check=True)
                if k0 + 1 == last:
                    epilogue_a(t, pz)
                    state["t"] = t + 1

            t = 0
            for g8 in range(0, TC, 8):
                khi = min(g8 + 8, TC)
                xs = xsp.tile([128, 2 * 128], BF16, tag="xs", name="xs")
                q0 = g8 // 4
                hi = min((q0 + 2) * 128, NQ * 128)
                nc.sync.dma_start(xs[:, :hi - q0 * 128],
                                  x4T_d[:, q0 * 128:hi])
                phs = []
                for k in range(g8, khi):
                    j, half = k % 4, (k - g8) // 4
                    ph = php.tile([128, 512], F32, tag="ph", name="ph")
                    nc.tensor.matmul(
                        ph[:, :HID],
                        lhsT=xs[32 * j:32 * j + 4,
                                half * 128:(half + 1) * 128],
                        rhs=wiT[32 * j:32 * j + 4, :],
                        start=True, stop=True, skip_group_check=True,
                        tile_position=(32 * j, 0))
                    phs.append(ph)
                # relus write chunk pairs into halves of a shared m2 tile;
                # even NCHUNK means pairs are (even k, k+1) and never span
                # a dest tile.
                for k in range(g8, khi):
                    ph = phs[k - g8]
                    if k % 2 == 0:
                        m2 = msgp.tile([128, 2 * HID], BF16, tag="m2",
                                       name="m2")
                        state["half"] = (m2, k)
                        nc.scalar.activation(m2[:, 0:HID], ph[:, :HID],
                                             AF.Relu, scale=v1[:, k:k + 1])
                    else:
                        m2, k0 = state["half"]
                        nc.vector.tensor_scalar(
                            m2[:, HID:2 * HID], ph[:, :HID],
                            v1[:, k:k + 1], 0.0,
                            op0=mybir.AluOpType.mult,
                            op1=mybir.AluOpType.max)
                        state["pend"].append((k0, m2))
                        state["half"] = None
                # flush pending pairs except those from the current group
                while len(state["pend"]) > 4:
                    flush_one()
            while state["pend"]:
                flush_one()
            while state["ag"] < len(cfg.PIECES):
                issue_ag(state["ag"])
                state["ag"] += 1

            ah_lo = ah_full[0:min(P, WIN), :]
            ah_hi = ah_full[cfg.HI_BASE:P, :]

            # ---- phase C: hop 2 (single-row gathers + S matmuls) ----
            ci = 0
            col0 = 0
            sk = 0
            for t in range(NT):
                ncht = int(C_lo[t]) + int(C_hi[t])
                pz = pzp.tile([128, 512], F32, tag="acc", name="acc")
                done = 0
                while done < ncht:
                    (tt, win, g) = callsC[ci]
                    assert tt == t
                    ni = g * 128
                    pr = pairp.tile([128, MAX_CALL_CHUNKS, HID], BF16,
                                    tag="pair", name="pair")
                    nc.gpsimd.dma_gather(
                        pr[:, :g, :], ah_lo if win == 0 else ah_hi,
                        idx[:, col0:col0 + ni // 16],
                        ni, ni, HID, single_packet=False)
                    sdl = msgp.tile([128, MAX_CALL_CHUNKS * 128], BF16,
                                    tag="sdl", name="sdl", bufs=3)
                    nc.sync.dma_start(sdl[:, :ni],
                                      sc_d[:, sk * 128:sk * 128 + ni])
                    for cc in range(g):
                        nc.tensor.matmul(
                            pz[:, :HID],
                            lhsT=sdl[:, cc * 128:(cc + 1) * 128],
                            rhs=pr[:, cc, :],
                            start=(done + cc == 0),
                            stop=(done + cc == ncht - 1),
                            skip_group_check=True)
                    done += g
                    sk += g
                    col0 += ni // 16
                    ci += 1
                a2b = stagep.tile([128, HID], BF16, tag="a2b", name="a2b")
                nc.vector.tensor_copy(a2b[:], pz[:, :HID])
                for mh in (0, 1):
                    pt = ptp.tile([128, 1024], BF16, tag="pt", name="pt")
                    nc.tensor.transpose(
                        pt[:, :128], a2b[:, mh * 128:(mh + 1) * 128],
                        idn16[:])
                    nc.vector.tensor_copy(
                        a2T[mh][:, t * 128:(t + 1) * 128], pt[:, :128])

            # ---- phase D: dense tail ----
            for b, (off, w) in enumerate(cfg.BLK):
                h2 = []
                for mh in (0, 1):
                    pd = pzp.tile([128, 512], F32, tag="acc", name="acc")
                    n = 0
                    for (wt, xt) in ((w1T, ahT), (w2T, a2T)):
                        for k in (0, 1):
                            nc.tensor.matmul(
                                pd[:, :w],
                                lhsT=wt[k][:, mh * 128:(mh + 1) * 128],
                                rhs=xt[k][:, off:off + w],
                                start=(n == 0), stop=(n == 3),
                                skip_group_check=True)
                            n += 1
                    ht = stagep.tile([128, 512], BF16, tag="h2t", name="h2t")
                    nc.scalar.activation(ht[:, :w], pd[:, :w], AF.Relu)
                    h2.append(ht)
                pg = php.tile([1, 512], F32, tag="ph", name="pg")
                for k in (0, 1):
                    nc.tensor.matmul(pg[:, :w],
                                     lhsT=woT[:, k:k + 1],
                                     rhs=h2[k][:, :w],
                                     start=(k == 0), stop=(k == 1),
                                     skip_group_check=True)
                gb = stagep.tile([1, 512], F32, tag="gbuf", name="gb",
                                 bufs=4)
                nc.vector.tensor_copy(gb[0:1, :w], pg[:, :w])
                ge = stagep.tile([1, 512], F32, tag="gbuf", name="ge",
                                 bufs=4)
                nc.scalar.activation(ge[0:1, :w], gb[0:1, :w], AF.Exp,
                                     bias=bout[0:1, :])
                go = stagep.tile([1, 512], F32, tag="gbuf", name="go",
                                 bufs=4)
                nc.scalar.activation(go[0:1, :w], ge[0:1, :w], AF.Ln,
                                     bias=1.0)
                nc.sync.dma_start(g_d[0:1, off:off + w], go[0:1, :w])
            if DEBUG_DUMP:
                nc.sync.dma_start(ahdump_d[:, :], ah_full[:, :])

    nc.compile()
    return nc


_COMPILED = {}


def _get_compiled(cfg, meta):
    key = (cfg.P, cfg.E, meta["NCHUNK"], meta["C_lo"], meta["C_hi"],
           meta["callsC"])
    if key not in _COMPILED:
        _COMPILED[key] = _build(cfg, meta)
    return _COMPILED[key]


def run(cfg, inputs, trace=False):
    per_core, consts, meta, (core_of, local_of) = _prepare(cfg, **inputs)
    ncobj = _get_compiled(cfg, meta)
    in_maps = []
    for c in range(cfg.NC):
        pc = per_core[c]
        im = {"x4t": pc["x4q"], "v1": pc["v1"], "sc": pc["sc"],
              "idx": pc["idx"]}
        im.update({k: np.asarray(v) for k, v in consts.items()})
        in_maps.append(im)
    res = run_bass_kernel_spmd(ncobj, in_maps, list(range(cfg.NC)),
                               trace=trace)
    g = np.empty(cfg.P, np.float32)
    for c in range(cfg.NC):
        go = np.asarray(res.results[c]["g"]).reshape(-1)
        mine = core_of == c
        g[mine] = go[local_of[mine]]
    return g.reshape(cfg.P, 1), res


def kernel(**inputs):
    cfg = Cfg(P=50000, E=800000)
    g, _ = run(cfg, inputs)
    return g


# revision 33
# speedup vs baseline: 1.1433x; 1.0883x over previous
"""BetaGNN message-passing kernel for 8 Trainium2 NeuronCores.

Strategy (dest-row sharding, 6250 nodes/core):
  - Host relabels nodes: sorted by in-degree, dealt round-robin to cores so
    every core's tile t has near-identical max-degree -> uniform chunk counts.
  - Hop 1 (AH = A @ relu(x @ W_in^T + b)): no gather. Host pre-gathers the
    3-wide input features per edge (plus a ones column for the bias); the PE
    recomputes h per edge-slot with one K=4 bf16 matmul per 128-edge chunk.
    Edge values fold into the relu via per-partition scale; constant-identity
    matmuls accumulate chunk PAIRS (N=512) into per-tile PSUM; the two
    halves are summed by DVE at tile end.
  - AH (bf16) is AllGathered in 4 strided pieces, pipelined under phase A.
  - Hop 2 (A2H = A @ AH): single-row dma_gather (512B packets). int16 index
    range is handled by splitting each tile's chunks into two source-windows
    ([0,32768) and [17232,50000)) with separate table base offsets. A host-
    built selection*value matrix S (one nonzero per slot row) is the lhsT of
    one N=256 matmul per chunk: psum[col,:] += sum_p S[p,col]*AH[src_p,:].
  - Dense tail in transposed layout (PE transposes AH/A2H tiles, bf16):
    h2^T = relu(W1 AH^T + W2 A2H^T), g = softplus(W_out h2^T + b_out).
"""

import sys

for _p in ("/opt/trn_rl_repo", "/root/.axon_site/_ro/trn_rl_repo"):
    if _p not in sys.path:
        sys.path.insert(0, _p)

import numpy as np
import ml_dtypes

import concourse.bacc as bacc
import concourse.bass as bass
import concourse.mybir as mybir
from concourse import tile
from concourse.bass_utils import run_bass_kernel_spmd
from concourse import bass_utils as _bu

# Enable walrus LDWEIGHTS dedup (identity/weight tiles reused between
# matmuls; the default =false flag forces a reload per matmul).
_orig_gwa = _bu.get_walrus_args
def _gwa(*a, **k):
    return [str(x).replace("--enable-ldw-opt=false", "--enable-ldw-opt=true")
            for x in _orig_gwa(*a, **k)]
_bu.get_walrus_args = _gwa

F32 = mybir.dt.float32
BF16 = mybir.dt.bfloat16
I16 = mybir.dt.int16
AF = mybir.ActivationFunctionType

DEBUG_DUMP = False
NSWQ = 1                  # SWDGE queues for gather parallelism
MAX_CALL_CHUNKS = 32      # <=32 chunks (4096 idxs) per dma_gather call
WIN = 32768               # int16-addressable window size
HI_BASE = None            # set per-P in Cfg (P - WIN, 0 if P <= WIN)


class Cfg:
    def __init__(self, P, E, nc=8, hid=256):
        assert P % (nc * 2) == 0
        self.P, self.E, self.NC, self.HID = P, E, nc, hid
        self.NPC = P // nc                    # nodes per core
        self.NT = (self.NPC + 127) // 128     # dest tiles per core
        self.NPAD = self.NT * 128
        self.HI_BASE = max(0, P - WIN)        # hi window = [HI_BASE, P)
        self.BLK = []
        off = 0
        while off < self.NPAD:
            w = min(512, self.NPAD - off)
            self.BLK.append((off, w))
            off += w
        # AllGather piece boundaries (in completed dest tiles). Each piece
        # writes a contiguous block of the table: rows off + c*R + (l - lo).
        npiece = 4 if self.NT >= 8 else 1
        step = (self.NT + npiece - 1) // npiece
        self.AG_AT = []
        b = step
        while b < self.NT:
            self.AG_AT.append(b)
            b += step
        self.AG_AT.append(self.NT)
        self.PIECES = []
        off = 0
        lo = 0
        for bnd in self.AG_AT:
            hi = min(bnd * 128, self.NPC)
            self.PIECES.append((lo, hi, off))
            off += nc * (hi - lo)
            lo = hi
        assert off == P


def _plan(cfg, deg):
    P, NC, NT = cfg.P, cfg.NC, cfg.NT
    order = np.argsort(-deg, kind="stable")
    rank = np.empty(P, np.int64)
    rank[order] = np.arange(P)
    core_of = rank % NC
    local_of = rank // NC
    gid = core_of * cfg.NPC + local_of
    degs_sorted = deg[order]
    NCHUNK = []
    for t in range(NT):
        q = max(2, int(degs_sorted[t * 128 * NC]))
        NCHUNK.append(q + (q & 1))   # even, so acc-matmul chunk pairs
    NCHUNK = np.array(NCHUNK, np.int64)
    tile_off = np.concatenate([[0], np.cumsum(NCHUNK)])
    return core_of, local_of, gid, NCHUNK, tile_off, int(tile_off[-1])


def _prepare(cfg, beta, degree, A_rows, A_cols, A_vals,
             W_in, b_in, W_mp1, W_mp2, W_out, b_out):
    P, E, NC, NPC, NT = cfg.P, cfg.E, cfg.NC, cfg.NPC, cfg.NT
    deg = np.bincount(A_rows, minlength=P).astype(np.int64)
    core_of, local_of, gid, NCHUNK, tile_off, TC = _plan(cfg, deg)
    NSLOT = TC * 128

    d_gid = gid[A_rows.astype(np.int64)]
    oe = np.argsort(d_gid, kind="stable")
    sd = d_gid[oe]
    first = np.r_[True, sd[1:] != sd[:-1]]
    cumstart = np.maximum.accumulate(np.where(first, np.arange(E), 0))
    chunk = np.arange(E) - cumstart
    e_core = sd // NPC
    e_local = sd % NPC
    e_col = e_local % 128
    e_k = tile_off[e_local // 128] + chunk
    e_slot = e_k * 128 + e_col

    src = A_cols.astype(np.int64)[oe]
    vals = A_vals[oe].astype(np.float32)
    # table row of each node: piece-major AllGather layout
    row_of_gid = np.empty(P, np.int64)
    for (lo, hi, off) in cfg.PIECES:
        R = hi - lo
        for c in range(NC):
            row_of_gid[c * NPC + lo:c * NPC + hi] = (
                off + c * R + np.arange(R))
    sgid = row_of_gid[gid[src]]

    x4_all = np.stack([beta[:, 0], beta[:, 0] ** 2, degree[:, 0],
                       np.ones(P, np.float32)], axis=0).astype(np.float32)

    # ---- phase C chunk planning: per (core, tile) window split ----
    HI_BASE = cfg.HI_BASE
    # per core/tile edge index lists
    et_tile = e_local // 128
    lo_strict = sgid < HI_BASE           # must use lo window
    hi_strict = sgid >= WIN              # must use hi window
    a_min = np.zeros((NC, NT), np.int64)
    b_min = np.zeros((NC, NT), np.int64)
    n_ct = np.zeros((NC, NT), np.int64)
    np.add.at(n_ct, (e_core, et_tile), 1)
    np.add.at(a_min, (e_core[lo_strict], et_tile[lo_strict]), 1)
    np.add.at(b_min, (e_core[hi_strict], et_tile[hi_strict]), 1)
    C_lo = np.max(-(-a_min // 128), axis=0)        # per-tile across-core max
    C_hi = np.max(-(-b_min // 128), axis=0)
    need = np.max(-(-n_ct // 128), axis=0)
    # ensure capacity C_lo+C_hi >= need per tile, and at least one chunk
    bump = np.maximum(0, need - (C_lo + C_hi))
    C_hi = C_hi + bump
    C_lo = np.maximum(C_lo + C_hi, 1) - C_hi       # C_lo+C_hi >= 1
    C_lo = C_lo.astype(np.int64)
    C_hi = C_hi.astype(np.int64)
    TCC = int(np.sum(C_lo + C_hi))
    NSLOTC = TCC * 128
    # chunk sequence: super-tile pairs, window-major within a pair, so
    # same-window chunks of adjacent tiles share one big gather call
    # (amortizes the ~1us fixed descriptor-gen cost per call) while only
    # two accumulator psums are ever alive.
    chunk_seq = []
    for st in range(0, NT, 2):
        pairt = [t for t in (st, st + 1) if t < NT]
        for win in (0, 1):
            for t in pairt:
                cnt = int(C_lo[t]) if win == 0 else int(C_hi[t])
                chunk_seq.extend([(t, win)] * cnt)
    callsC = []
    i = 0
    while i < len(chunk_seq):
        w = chunk_seq[i][1]
        j = i
        while (j < len(chunk_seq) and chunk_seq[j][1] == w
               and j - i < MAX_CALL_CHUNKS):
            j += 1
        segs = []
        for (t, _) in chunk_seq[i:j]:
            if segs and segs[-1][0] == t:
                segs[-1][1] += 1
            else:
                segs.append([t, 1])
        callsC.append((w, tuple((t, c) for t, c in segs)))
        i = j
    callsC = tuple(callsC)

    NIDXCOL = NSLOTC // 16
    per_core = []
    for c in range(NC):
        m = e_core == c
        # ---- phase A tensors (x4 quad-packed + v1), as baseline ----
        sl = e_slot[m]
        x4T = np.zeros((4, NSLOT), np.float32)
        # fold the (positive) edge value into the features: relu(v*z)=v*relu(z)
        x4T[:, sl] = x4_all[:, src[m]] * vals[m]
        NQ = (TC + 3) // 4
        x4c = np.zeros((4, NQ * 4, 128), np.float32)
        x4c[:, :TC, :] = x4T.reshape(4, TC, 128)
        x4q = np.zeros((128, NQ * 128), np.float32)
        for j in range(4):
            x4q[32 * j:32 * j + 4, :] = (
                x4c[:, j::4, :].reshape(4, NQ * 128))
        # ---- phase C: window assignment, slots, S, idx ----
        tt_c = et_tile[m]
        sg_c = sgid[m]
        col_c = e_col[m]
        val_c = vals[m]
        idx_slot = np.zeros(NSLOTC, np.int16)
        s_mat = np.zeros((128, TCC, 128), ml_dtypes.bfloat16)
        # per (tile, window): edge arrays after mid-assignment
        win_edges = {}
        for t in range(NT):
            sel = tt_c == t
            sg_t, col_t, val_t = sg_c[sel], col_c[sel], val_c[sel]
            n = len(sg_t)
            is_hi_strict = sg_t >= WIN
            is_lo_strict = sg_t < HI_BASE
            is_mid = ~is_hi_strict & ~is_lo_strict
            bm = int(np.sum(is_hi_strict))
            b = max(bm, n - int(C_lo[t]) * 128)
            a = n - b
            amin = int(np.sum(is_lo_strict))
            nmid_lo = a - amin
            mid_idx = np.nonzero(is_mid)[0]
            lo_sel = np.zeros(n, bool)
            lo_sel[is_lo_strict] = True
            lo_sel[mid_idx[:nmid_lo]] = True
            assert a <= int(C_lo[t]) * 128 and b <= int(C_hi[t]) * 128
            win_edges[(t, 0)] = (sg_t[lo_sel], col_t[lo_sel], val_t[lo_sel])
            win_edges[(t, 1)] = (sg_t[~lo_sel] - HI_BASE, col_t[~lo_sel],
                                 val_t[~lo_sel])
        cur = {k: 0 for k in win_edges}
        for k, (t, win) in enumerate(chunk_seq):
            sg_t, col_t, val_t = win_edges[(t, win)]
            c0 = cur[(t, win)]
            nw = min(128, len(sg_t) - c0)
            cur[(t, win)] = c0 + 128
            if nw <= 0:
                continue
            idx_slot[k * 128:k * 128 + nw] = sg_t[c0:c0 + nw].astype(np.int16)
            s_mat[np.arange(nw), k, col_t[c0:c0 + nw]] = (
                val_t[c0:c0 + nw].astype(ml_dtypes.bfloat16))
        # pack indices per call ([16, ni/16] wrap, replicated x8)
        idxh = np.zeros((128, NIDXCOL), np.int16)
        col0 = 0
        soff = 0
        for (win, segs) in callsC:
            ni = sum(c for _, c in segs) * 128
            blockv = idx_slot[soff:soff + ni].reshape(ni // 16, 16).T
            for q in range(8):
                idxh[16 * q:16 * (q + 1), col0:col0 + ni // 16] = blockv
            col0 += ni // 16
            soff += ni
        per_core.append(dict(
            x4q=x4q.astype(ml_dtypes.bfloat16),
            sc=s_mat.reshape(128, TCC * 128),
            idx=idxh))

    wiT = np.concatenate([W_in.T.astype(np.float32),
                          b_in[None, :].astype(np.float32)], axis=0)
    wiT4 = np.zeros((128, wiT.shape[1]), np.float32)
    for j in range(4):
        wiT4[32 * j:32 * j + 4, :] = wiT
    consts = dict(
        wit=wiT4.astype(ml_dtypes.bfloat16),
        w1t=np.ascontiguousarray(W_mp1.T).astype(ml_dtypes.bfloat16),
        w2t=np.ascontiguousarray(W_mp2.T).astype(ml_dtypes.bfloat16),
        wot=np.ascontiguousarray(W_out.T).astype(ml_dtypes.bfloat16),
        bout=np.full((128, 1), float(np.asarray(b_out).reshape(-1)[0]),
                     np.float32),
        idn16=np.eye(128, dtype=np.float32).astype(ml_dtypes.bfloat16),
    )
    meta = dict(NCHUNK=tuple(int(x) for x in NCHUNK),
                C_lo=tuple(int(x) for x in C_lo),
                C_hi=tuple(int(x) for x in C_hi),
                callsC=tuple(callsC),
                TC=TC, TCC=TCC, NIDXCOL=NIDXCOL, NQ=(TC + 3) // 4)
    return per_core, consts, meta, (core_of, local_of)


def _build(cfg, meta):
    NT, NPC, NPAD, HID, NC, P = (cfg.NT, cfg.NPC, cfg.NPAD, cfg.HID,
                                 cfg.NC, cfg.P)
    NCHUNK = meta["NCHUNK"]
    C_lo, C_hi, callsC = meta["C_lo"], meta["C_hi"], meta["callsC"]
    TC, TCC, NIDXCOL, NQ = meta["TC"], meta["TCC"], meta["NIDXCOL"], meta["NQ"]
    tile_off = np.concatenate([[0], np.cumsum(NCHUNK)])
    NBLK = len(cfg.BLK)

    nc = bacc.Bacc("TRN2", target_bir_lowering=False, debug=False,
               num_swdge_queues=NSWQ)
    x4T_d = nc.dram_tensor("x4t", [128, NQ * 128], BF16, kind="ExternalInput")
    sc_d = nc.dram_tensor("sc", [128, TCC * 128], BF16, kind="ExternalInput")
    idx_d = nc.dram_tensor("idx", [128, NIDXCOL], I16, kind="ExternalInput")
    wiT_d = nc.dram_tensor("wit", [128, HID], BF16, kind="ExternalInput")
    w1T_d = nc.dram_tensor("w1t", [HID, HID], BF16, kind="ExternalInput")
    w2T_d = nc.dram_tensor("w2t", [HID, HID], BF16, kind="ExternalInput")
    woT_d = nc.dram_tensor("wot", [HID, 1], BF16, kind="ExternalInput")
    bout_d = nc.dram_tensor("bout", [128, 1], F32, kind="ExternalInput")
    idn16_d = nc.dram_tensor("idn16", [128, 128], BF16, kind="ExternalInput")
    g_d = nc.dram_tensor("g", [1, NBLK * 512], F32, kind="ExternalOutput")
    ahdump_d = (nc.dram_tensor("ahdump", [P, HID], BF16,
                kind="ExternalOutput") if DEBUG_DUMP else None)

    ah_bounce = nc.dram_tensor("ah_bounce", [NPC, HID], BF16)
    ah_full = nc.dram_tensor("ah_full", [P, HID], BF16, addr_space="Shared")

    with tile.TileContext(nc) as tc:
        with (
            tc.tile_pool(name="const", bufs=1) as constp,
            tc.tile_pool(name="xs", bufs=3) as xsp,
            tc.tile_pool(name="msgs", bufs=12) as msgp,
            tc.tile_pool(name="stage", bufs=3) as stagep,
            tc.tile_pool(name="resid", bufs=1) as residp,
            tc.tile_pool(name="pair", bufs=3) as pairp,
            tc.tile_pool(name="ph", bufs=4, space="PSUM") as php,
            tc.tile_pool(name="pz", bufs=2, space="PSUM") as pzp,
            tc.tile_pool(name="pt", bufs=1, space="PSUM") as ptp,
        ):
            wiT = constp.tile([128, HID], BF16, tag="wiT", name="wiT")
            nc.sync.dma_start(wiT[:], wiT_d[:])
            w1T = [constp.tile([128, HID], BF16, tag=f"w1_{k}", name=f"w1_{k}")
                   for k in (0, 1)]
            w2T = [constp.tile([128, HID], BF16, tag=f"w2_{k}", name=f"w2_{k}")
                   for k in (0, 1)]
            for k in (0, 1):
                nc.sync.dma_start(w1T[k][:], w1T_d[128 * k:128 * (k + 1), :])
                nc.sync.dma_start(w2T[k][:], w2T_d[128 * k:128 * (k + 1), :])
            woT = constp.tile([128, 2], BF16, tag="woT", name="woT")
            nc.sync.dma_start(woT[:, 0:1], woT_d[0:128, :])
            nc.sync.dma_start(woT[:, 1:2], woT_d[128:256, :])
            bout = constp.tile([128, 1], F32, tag="bout", name="bout")
            nc.sync.dma_start(bout[:], bout_d[:])
            idn16 = constp.tile([128, 128], BF16, tag="idn16", name="idn16")
            nc.sync.dma_start(idn16[:], idn16_d[:])
            idx = constp.tile([128, NIDXCOL], I16, tag="idx", name="idx")
            nc.sync.dma_start(idx[:], idx_d[:])

            ahT = [residp.tile([128, NPAD], BF16, tag=f"ahT{m}", name=f"ahT{m}")
                   for m in (0, 1)]
            a2T = [residp.tile([128, NPAD], BF16, tag=f"a2T{m}", name=f"a2T{m}")
                   for m in (0, 1)]

            def issue_ag(piece):
                # AllGather local rows [lo, hi) into the contiguous table
                # block [off, off + NC*(hi-lo)): replica c lands at off + c*R.
                (lo, hi, off) = cfg.PIECES[piece]
                R = hi - lo
                nc.gpsimd.collective_compute(
                    "AllGather", mybir.AluOpType.bypass,
                    replica_groups=[list(range(NC))],
                    ins=[ah_bounce[lo:hi, :]],
                    outs=[ah_full[off:off + NC * R, :]],
                )

            # ---- phase A: hop 1 (quad-packed K=4 matmuls, groups of 8) ----
            # software pipeline: acc matmuls run one group behind the
            # h-matmuls so relu latency is hidden.
            state = dict(t=0, pz=None, pend=[], half=None, ag=0)

            def epilogue_a(tt, pzv):
                # combine pair halves, emit bf16 AH tile + transposes
                tmp = stagep.tile([128, HID], BF16, tag="tmp", name="tmp")
                nc.vector.tensor_copy(tmp[:], pzv[:, HID:2 * HID])
                ahb = stagep.tile([128, HID], BF16, tag="ahb", name="ahb")
                nc.vector.tensor_tensor(
                    ahb[:], pzv[:, :HID], tmp[:],
                    op=mybir.AluOpType.add)
                rows = min(128, NPC - tt * 128)
                nc.sync.dma_start(ah_bounce[tt * 128:tt * 128 + rows, :],
                                  ahb[:rows, :])
                for mh in (0, 1):
                    pt = ptp.tile([128, 1024], BF16, tag="pt", name="pt")
                    nc.tensor.transpose(
                        pt[:, :128], ahb[:, mh * 128:(mh + 1) * 128],
                        idn16[:])
                    nc.vector.tensor_copy(
                        ahT[mh][:, tt * 128:(tt + 1) * 128], pt[:, :128])
                for j, bnd in enumerate(cfg.AG_AT):
                    if tt + 1 == bnd:
                        issue_ag(j)
                        state["ag"] = j + 1

            def flush_one():
                # consume one pending chunk-pair into the accumulator psum;
                # advance tile state. NCHUNK is even so pairs never span
                # tiles and both psum halves are always started/stopped.
                k0, m2 = state["pend"].pop(0)
                t = state["t"]
                if k0 == int(tile_off[t]):
                    state["pz"] = pzp.tile([128, 512], F32, tag="acc",
                                           name="acc")
                pz = state["pz"]
                last = int(tile_off[t + 1]) - 1
                nc.tensor.matmul(
                    pz[:], lhsT=idn16[:], rhs=m2[:],
                    start=(k0 == int(tile_off[t])),
                    stop=(k0 + 1 == last),
                    skip_group_check=True)
                if k0 + 1 == last:
                    epilogue_a(t, pz)
                    state["t"] = t + 1

            t = 0
            for g8 in range(0, TC, 8):
                khi = min(g8 + 8, TC)
                xs = xsp.tile([128, 2 * 128], BF16, tag="xs", name="xs")
                q0 = g8 // 4
                hi = min((q0 + 2) * 128, NQ * 128)
                nc.sync.dma_start(xs[:, :hi - q0 * 128],
                                  x4T_d[:, q0 * 128:hi])
                phs = []
                for k in range(g8, khi):
                    j, half = k % 4, (k - g8) // 4
                    ph = php.tile([128, 512], F32, tag="ph", name="ph")
                    nc.tensor.matmul(
                        ph[:, :HID],
                        lhsT=xs[32 * j:32 * j + 4,
                                half * 128:(half + 1) * 128],
                        rhs=wiT[32 * j:32 * j + 4, :],
                        start=True, stop=True, skip_group_check=True,
                        tile_position=(32 * j, 0))
                    phs.append(ph)
                # relus (plain: edge values pre-folded into the features)
                # write chunk pairs into halves of a shared m2 tile,
                # alternating between the scalar and vector engines.
                for k in range(g8, khi):
                    ph = phs[k - g8]
                    if k % 2 == 0:
                        m2 = msgp.tile([128, 2 * HID], BF16, tag="m2",
                                       name="m2")
                        state["half"] = (m2, k)
                        nc.scalar.activation(m2[:, 0:HID], ph[:, :HID],
                                             AF.Relu)
                    else:
                        m2, k0 = state["half"]
                        nc.vector.tensor_scalar(
                            m2[:, HID:2 * HID], ph[:, :HID], 1.0, 0.0,
                            op0=mybir.AluOpType.mult,
                            op1=mybir.AluOpType.max)
                        state["pend"].append((k0, m2))
                        state["half"] = None
                # flush pending pairs except those from the current group
                while len(state["pend"]) > 4:
                    flush_one()
            while state["pend"]:
                flush_one()
            while state["ag"] < len(cfg.PIECES):
                issue_ag(state["ag"])
                state["ag"] += 1

            ah_lo = ah_full[0:min(P, WIN), :]
            ah_hi = ah_full[cfg.HI_BASE:P, :]

            # ---- phase D block emitter (interleaved into phase C) ----
            def emit_block(b):
                (off, w) = cfg.BLK[b]
                h2 = []
                for mh in (0, 1):
                    pd = php.tile([128, 512], F32, tag="ph", name="pd")
                    n = 0
                    for (wt, xt) in ((w1T, ahT), (w2T, a2T)):
                        for k in (0, 1):
                            nc.tensor.matmul(
                                pd[:, :w],
                                lhsT=wt[k][:, mh * 128:(mh + 1) * 128],
                                rhs=xt[k][:, off:off + w],
                                start=(n == 0), stop=(n == 3),
                                skip_group_check=True)
                            n += 1
                    ht = stagep.tile([128, 512], BF16, tag="h2t", name="h2t")
                    nc.scalar.activation(ht[:, :w], pd[:, :w], AF.Relu)
                    h2.append(ht)
                pg = php.tile([1, 512], F32, tag="pg", name="pg", bufs=1)
                for k in (0, 1):
                    nc.tensor.matmul(pg[:, :w],
                                     lhsT=woT[:, k:k + 1],
                                     rhs=h2[k][:, :w],
                                     start=(k == 0), stop=(k == 1),
                                     skip_group_check=True)
                gb = stagep.tile([1, 512], F32, tag="gbuf", name="gb",
                                 bufs=4)
                nc.vector.tensor_copy(gb[0:1, :w], pg[:, :w])
                ge = stagep.tile([1, 512], F32, tag="gbuf", name="ge",
                                 bufs=4)
                nc.scalar.activation(ge[0:1, :w], gb[0:1, :w], AF.Exp,
                                     bias=bout[0:1, :])
                go = stagep.tile([1, 512], F32, tag="gbuf", name="go",
                                 bufs=4)
                nc.scalar.activation(go[0:1, :w], ge[0:1, :w], AF.Ln,
                                     bias=1.0)
                nc.sync.dma_start(g_d[0:1, off:off + w], go[0:1, :w])

            # ---- phase C: hop 2 (single-row cross-tile gathers + S
            # matmuls), with phase-D blocks emitted as their a2T tiles
            # complete ----
            total = {t: int(C_lo[t]) + int(C_hi[t]) for t in range(NT)}
            done = {t: 0 for t in range(NT)}
            pzs = {}
            tiles_done = 0
            next_blk = 0
            col0 = 0
            sk = 0
            for (win, segs) in callsC:
                g = sum(c for _, c in segs)
                ni = g * 128
                pr = pairp.tile([128, MAX_CALL_CHUNKS, HID], BF16,
                                tag="pair", name="pair")
                nc.gpsimd.dma_gather(
                    pr[:, :g, :], ah_lo if win == 0 else ah_hi,
                    idx[:, col0:col0 + ni // 16],
                    ni, ni, HID, single_packet=False)
                sdl = msgp.tile([128, MAX_CALL_CHUNKS * 128], BF16,
                                tag="sdl", name="sdl", bufs=3)
                nc.sync.dma_start(sdl[:, :ni],
                                  sc_d[:, sk * 128:sk * 128 + ni])
                cc = 0
                for (t, cnt) in segs:
                    if done[t] == 0:
                        pzs[t] = pzp.tile([128, 512], F32, tag="acc",
                                          name="acc")
                    pz = pzs[t]
                    for j in range(cnt):
                        nc.tensor.matmul(
                            pz[:, :HID],
                            lhsT=sdl[:, cc * 128:(cc + 1) * 128],
                            rhs=pr[:, cc, :],
                            start=(done[t] == 0),
                            stop=(done[t] == total[t] - 1),
                            skip_group_check=True)
                        done[t] += 1
                        cc += 1
                    if done[t] == total[t]:
                        a2b = stagep.tile([128, HID], BF16, tag="a2b",
                                          name="a2b")
                        nc.vector.tensor_copy(a2b[:], pz[:, :HID])
                        del pzs[t]
                        for mh in (0, 1):
                            pt = ptp.tile([128, 1024], BF16, tag="pt",
                                          name="pt")
                            nc.tensor.transpose(
                                pt[:, :128], a2b[:, mh * 128:(mh + 1) * 128],
                                idn16[:])
                            nc.vector.tensor_copy(
                                a2T[mh][:, t * 128:(t + 1) * 128],
                                pt[:, :128])
                        tiles_done += 1
                        while (next_blk < NBLK and
                               cfg.BLK[next_blk][0] + cfg.BLK[next_blk][1]
                               <= tiles_done * 128):
                            emit_block(next_blk)
                            next_blk += 1
                sk += g
                col0 += ni // 16
            while next_blk < NBLK:
                emit_block(next_blk)
                next_blk += 1
            if DEBUG_DUMP:
                nc.sync.dma_start(ahdump_d[:, :], ah_full[:, :])

    nc.compile()
    return nc


_COMPILED = {}


def _get_compiled(cfg, meta):
    key = (cfg.P, cfg.E, meta["NCHUNK"], meta["C_lo"], meta["C_hi"],
           meta["callsC"])
    if key not in _COMPILED:
        _COMPILED[key] = _build(cfg, meta)
    return _COMPILED[key]


def run(cfg, inputs, trace=False):
    per_core, consts, meta, (core_of, local_of) = _prepare(cfg, **inputs)
    ncobj = _get_compiled(cfg, meta)
    in_maps = []
    for c in range(cfg.NC):
        pc = per_core[c]
        im = {"x4t": pc["x4q"], "sc": pc["sc"], "idx": pc["idx"]}
        im.update({k: np.asarray(v) for k, v in consts.items()})
        in_maps.append(im)
    res = run_bass_kernel_spmd(ncobj, in_maps, list(range(cfg.NC)),
                               trace=trace)
    g = np.empty(cfg.P, np.float32)
    for c in range(cfg.NC):
        go = np.asarray(res.results[c]["g"]).reshape(-1)
        mine = core_of == c
        g[mine] = go[local_of[mine]]
    return g.reshape(cfg.P, 1), res


def kernel(**inputs):
    cfg = Cfg(P=50000, E=800000)
    g, _ = run(cfg, inputs)
    return g
